# revision 1
# baseline (speedup 1.0000x reference)
"""Trainium2 Bass kernel for nn_AllLoss_13400297964003.

Strategy (exact algebraic refactor of the reference loss):
  - The mask BCE term per anchor m is
        mean_{512x512}( softplus(up) - goal*up )
    with up = 4x nearest-upsample of z_m = coef_m . proto.  This equals
        ( 16*sum_ij softplus(z_m[ij]) - sum_ij z_m[ij]*G_m[ij] ) / 512^2
    where G_m = 4x4 block-sum pooling of gt_masks[gt_idx[m]].
  - The goal term collapses:  sum_m sum_ij z_m*G_m = sum_{p,g} C[p,g]*D[p,g]
    with C[p,g] = sum_{m: gt_idx[m]=g} coef[m,p]  (tiny, host-aggregated)
    and  D[p,g] = sum_ij proto[p,ij] * pool4x4(mask_g)[ij]  (device).
  - Sharding over 8 cores: core c gets anchors [32c,32c+32), gt masks
    [4c,4c+4), and 96 negative anchors.  Each core reads 4.2MB of masks
    (a perfect shard of the 33.5MB dominant input), computes partial sums,
    host combines scalars in float64.

Device work per core:
  - z via bf16 matmuls: block-diag weights [16,128] x proto16 [16,4096]
    (one merged 'zin' DMA) -> z in PSUM, full partition occupancy.
    softplus = Exp then Ln(bias=1) on ACT with accum_out; all Exp-set
    ops are chain-ordered before all Ln-set ops so the ACT spline
    tables load exactly twice (they thrash otherwise, ~1.5us each).
  - mask 4x4 pooling: row-pool via float32r matmuls with constant 0/1
    matrices (exact for 0/1 masks; f32r streams 1 col/cycle vs 4 for
    f32), column-pool via one strided tensor_reduce per mask.
  - D partials via DVE multiply + segmented reduce in bf16 (pooled
    sums <=16 are exact in bf16).
  - cls/loc losses as packed 128-row columns (gathers done host-side,
    all arithmetic incl. log10/reciprocal/smooth-L1 on device).
  - DMA plan: the 16 mask chunks ride the gpsimd SWDGE queue; the five
    small const DMAs get exclusive HWDGE lanes (sync+scalar) so their
    completion semaphores are never queued behind mask packets (Tile
    serializes sem-lane reuse; mixing them cost ~5us of engine stall).
"""

import numpy as np

N_CORES = 8
M = 256
NUM_GT = 32
M_LOC = M // N_CORES          # 32 anchors per core
G_LOC = NUM_GT // N_CORES     # 4 gt masks per core
NEG_LOC = 3 * M // N_CORES    # 96 negative anchors per core
LN10 = float(np.log(10.0))
NCOL = 20                     # result cols: 2 soft, 1 cls, 1 loc, 16 ddot

_CACHE = {}


def _build_nc():
    from contextlib import ExitStack
    import concourse.tile as tile
    from concourse import bacc, mybir
    from concourse.tile import add_dep_helper

    f32 = mybir.dt.float32
    f32r = mybir.dt.float32r
    bf16 = mybir.dt.bfloat16
    AF = mybir.ActivationFunctionType
    ALU = mybir.AluOpType
    AX = mybir.AxisListType

    nc = bacc.Bacc("TRN2", target_bir_lowering=False, debug=False)

    masks = nc.dram_tensor("masks", [G_LOC, 512, 512], f32r, kind="ExternalInput").ap()
    zin = nc.dram_tensor("zin", [16, 4224], bf16, kind="ExternalInput").ap()
    proto_cat = nc.dram_tensor("proto_cat", [128, 512], bf16, kind="ExternalInput").ap()
    small8 = nc.dram_tensor("small8", [128, 8], f32, kind="ExternalInput").ap()
    res = nc.dram_tensor("res", [128, NCOL], f32, kind="ExternalOutput").ap()

    with tile.TileContext(nc) as tc:
        with ExitStack() as ctx:
            constp = ctx.enter_context(tc.tile_pool(name="constp", bufs=1))
            maskp = ctx.enter_context(tc.tile_pool(name="maskp", bufs=8))
            zps = ctx.enter_context(tc.tile_pool(name="zps", bufs=3, space="PSUM"))
            rps = ctx.enter_context(tc.tile_pool(name="rps", bufs=2, space="PSUM"))
            exps = ctx.enter_context(tc.tile_pool(name="exps", bufs=1))
            workp = ctx.enter_context(tc.tile_pool(name="workp", bufs=3))
            outp = ctx.enter_context(tc.tile_pool(name="outp", bufs=1))

            # ---- constant / small input loads ----
            zin_t = constp.tile([16, 4224], bf16)
            nc.scalar.dma_start(zin_t[:], zin[:])
            proto16_t = zin_t[:, 0:4096]
            w16_t = zin_t[:, 4096:4224]
            sr_t = constp.tile([128, 512], f32r)
            small8_t = constp.tile([128, 8], f32)
            nc.sync.dma_start(small8_t[:], small8[:])
            proto_cat_t = constp.tile([128, 512], bf16)
            nc.scalar.dma_start(proto_cat_t[:], proto_cat[:])
            clsx_t = small8_t[:, 0:1]
            clssgn_t = small8_t[:, 1:2]
            locp_t = small8_t[:, 2:3]
            locu_t = small8_t[:, 3:4]
            locv_t = small8_t[:, 4:5]
            locw_t = small8_t[:, 5:6]

            PS = outp.tile([128, NCOL], f32)

            exp_phase = []   # ACT ops using the Exp table set (+ fillers)
            ln_phase = []    # ACT ops using the Ln table set

            # ---- z matmuls (bf16) + softplus ----
            z_mms = []
            exm = exps.tile([128, 4096], f32, tag="ex")
            for b in range(4):
                zt = zps.tile([128, 1024], f32, tag="z")
                z_mms.append(nc.tensor.matmul(
                    zt[:, 0:512], w16_t,
                    proto16_t[:, 1024 * b:1024 * b + 512],
                    start=True, stop=True))
                z_mms.append(nc.tensor.matmul(
                    zt[:, 512:1024], w16_t,
                    proto16_t[:, 1024 * b + 512:1024 * (b + 1)],
                    start=True, stop=True))
                exp_phase.append(nc.scalar.activation(
                    exm[:, 1024 * b:1024 * (b + 1)], zt[:], AF.Exp))

            # ---- cls/loc ACT ops, grouped into the two table phases ----
            et = workp.tile([128, 1], f32, tag="sm1")
            exp_phase.append(
                nc.scalar.activation(et[:], clsx_t, AF.Exp, scale=clssgn_t))
            fu = workp.tile([128, 1], f32, tag="sm2")
            exp_phase.append(
                nc.scalar.activation(fu[0:64, :], locu_t[0:64, :], AF.Identity))
            fv = workp.tile([128, 1], f32, tag="sm3")
            exp_phase.append(
                nc.scalar.activation(fv[0:64, :], locv_t[0:64, :], AF.Identity))

            ln_phase.append(
                nc.scalar.activation(PS[:, 2:3], et[:], AF.Ln, bias=1.0))
            ln_phase.append(
                nc.scalar.activation(fu[64:128, :], locu_t[64:128, :], AF.Ln))
            ln_phase.append(
                nc.scalar.activation(fv[64:128, :], locv_t[64:128, :], AF.Ln))
            for b in range(2):
                ln_phase.append(nc.scalar.activation(
                    exm[:, 2048 * b:2048 * (b + 1)],
                    exm[:, 2048 * b:2048 * (b + 1)], AF.Ln,
                    bias=1.0, accum_out=PS[:, b:b + 1]))

            # chain the ACT program order: all Exp-set ops, then all Ln-set ops
            order = exp_phase + ln_phase
            for a, b2 in zip(order, order[1:]):
                add_dep_helper(b2.ins, a.ins, sync=False, reason="act-table-phase")

            # ---- mask chunk DMAs (the dominant traffic) ----
            chunk = {}
            pairs = [(g, c) for g in range(G_LOC) for c in (0, 2)]

            def issue_pair(g, c):
                t = maskp.tile([128, 1024], f32r, tag="mask")
                src_ap = masks[g, 128 * c:128 * (c + 2), :].rearrange(
                    "(h i) J -> i h J", h=2)
                nc.gpsimd.dma_start(t[:], src_ap)
                chunk[(g, c)] = t[:, 0:512]
                chunk[(g, c + 1)] = t[:, 512:1024]

            for g, c in pairs[:2]:
                issue_pair(g, c)

            # ---- generate the 0/1 row-pool matrix on-device ----
            # sr[I, 128c+k] = 1 iff k == 32c + I//4, i.e. (4k-128c-I) in [-3,0].
            # The ones source is a framework const AP (no DMA dependency), so
            # this runs as soon as the gpsimd sequencer reaches it.
            srt1 = workp.tile([128, 4, 128], f32r, tag="srg")
            ones_col = nc.const_aps.scalar_like(1.0, srt1[:, 0, 0:1])
            ones_b = ones_col.broadcast_to([128, 4, 128])
            nc.gpsimd.affine_select(
                srt1[:], ones_b, pattern=[[128, 4], [-4, 128]],
                compare_op=ALU.is_ge, fill=0.0, base=0, channel_multiplier=1)
            sr3 = sr_t[:].rearrange("p (c k) -> p c k", c=4)
            nc.gpsimd.affine_select(
                sr3, srt1[:], pattern=[[-128, 4], [4, 128]],
                compare_op=ALU.is_ge, fill=0.0, base=3, channel_multiplier=-1)

            for g, c in pairs[2:]:
                issue_pair(g, c)


            # ---- mask pooling + D partials ----
            dve_order = []
            pc3 = proto_cat_t[:].rearrange("p (a k) -> p a k", a=4)
            for g in range(G_LOC):
                R = rps.tile([128, 512], f32, tag="r")
                for c in range(4):
                    mm = nc.tensor.matmul(
                        R[:],
                        sr_t[:, 128 * c:128 * (c + 1)],
                        chunk[(g, c)],
                        start=(c == 0), stop=(c == 3),
                    )
                    add_dep_helper(mm.ins, z_mms[-1].ins, sync=False,
                                   reason="z-first")
                r4 = R[:].rearrange("p (j four) -> p j four", four=4)
                Pg = workp.tile([128, 128], bf16, tag="Pg")
                with nc.allow_low_precision(
                        reason="pooled 0/1 mask sums <=16 are exact in bf16"):
                    dve_order.append(
                        nc.vector.tensor_reduce(Pg[:], r4, axis=AX.X, op=ALU.add))
                prod = workp.tile([128, 4, 128], bf16, tag="prod")
                pgb = Pg[:].unsqueeze(1).broadcast_to([128, 4, 128])
                dve_order.append(nc.vector.tensor_mul(prod[:], pgb, pc3))
                dve_order.append(
                    nc.vector.tensor_reduce(PS[:, 4 + 4 * g:8 + 4 * g], prod[:],
                                            axis=AX.X, op=ALU.add))

            # ---- localization smooth-L1 column ----
            rw = workp.tile([128, 1], f32, tag="sm4")
            nc.vector.reciprocal(rw[:], locw_t)
            for a, b2 in zip(dve_order, dve_order[1:]):
                add_dep_helper(b2.ins, a.ins, sync=False, reason="dve-order")
            last_pool = dve_order[-1]
            df = workp.tile([128, 1], f32, tag="sm5")
            df_i = nc.vector.tensor_sub(df[:], fu[:], fv[:])
            add_dep_helper(df_i.ins, dve_order[5].ins, sync=False, reason="loc-mid")
            tgt = workp.tile([128, 1], f32, tag="sm6")
            nc.vector.tensor_mul(tgt[:], df[:], rw[:])
            d = workp.tile([128, 1], f32, tag="sm7")
            nc.vector.tensor_sub(d[:], locp_t, tgt[:])
            a_t = workp.tile([128, 1], f32, tag="sm8")
            nc.scalar.activation(a_t[:], d[:], AF.Abs)
            mn = workp.tile([128, 1], f32, tag="sm9")
            nc.vector.tensor_scalar(mn[:], a_t[:], 1.0, None, op0=ALU.min)
            amn = workp.tile([128, 1], f32, tag="sm10")
            nc.vector.tensor_sub(amn[:], a_t[:], mn[:])
            sq = workp.tile([128, 1], f32, tag="sm11")
            nc.vector.tensor_mul(sq[:], mn[:], mn[:])
            nc.vector.scalar_tensor_tensor(PS[:, 3:4], sq[:], 0.5, amn[:],
                                           op0=ALU.mult, op1=ALU.add)

            # ---- write result ----
            nc.sync.dma_start(res[:], PS[:])

    nc.compile()
    return nc


def _get_nc():
    if "nc" not in _CACHE:
        _CACHE["nc"] = _build_nc()
    return _CACHE["nc"]


def _host_prep(inputs):
    """Pure index-driven gathers/packing. Returns per-core input maps plus
    the float64 C aggregation matrix used in the final scalar combine."""
    import ml_dtypes
    bf16 = ml_dtypes.bfloat16
    f32 = np.float32
    proto = np.asarray(inputs["proto_types"], f32)[0]        # (4,128,128)
    map_class = np.asarray(inputs["map_class"], f32)[0]      # (3,64,64)
    map_box = np.asarray(inputs["map_box"], f32)[0]          # (12,64,64)
    map_coef = np.asarray(inputs["map_coef"], f32)[0]        # (12,64,64)
    anchor_center = np.asarray(inputs["anchor_center"], f32)  # (2,64,64)
    anchor_box = np.asarray(inputs["anchor_box"], f32)       # (3,2)
    gt_boxes = np.asarray(inputs["gt_boxes"], f32)[0]        # (32,4)
    gt_masks = np.asarray(inputs["gt_masks"], f32)[0]        # (32,512,512)
    pos_idx = np.asarray(inputs["pos_idx"])
    gt_idx = np.asarray(inputs["gt_idx"])
    neg_idx = np.asarray(inputs["neg_idx"])

    r, hh, ww = pos_idx[:, 0], pos_idx[:, 1], pos_idx[:, 2]
    ch4 = r[:, None] * 4 + np.arange(4, dtype=r.dtype)[None, :]
    coef = map_coef[ch4, hh[:, None], ww[:, None]]           # (256,4)
    pred = map_box[ch4, hh[:, None], ww[:, None]]            # (256,4)
    logit_pos = map_class[r, hh, ww]                         # (256,)
    logit_neg = map_class[neg_idx[:, 0], neg_idx[:, 1], neg_idx[:, 2]]  # (768,)
    a_ch = anchor_center[0, hh, ww]
    a_cw = anchor_center[1, hh, ww]
    a_h = anchor_box[r, 0]
    a_w = anchor_box[r, 1]
    gt = gt_boxes[gt_idx]                                    # (256,4)

    # replicated tensors
    proto_flat = proto.reshape(4, 16384)
    proto16 = np.ascontiguousarray(
        proto_flat.reshape(4, 4, 4096).transpose(1, 0, 2).reshape(16, 4096)
    ).astype(bf16)
    proto_cat = np.ascontiguousarray(proto.transpose(1, 0, 2).reshape(128, 512)).astype(bf16)
    # C[p,g] aggregation (float64, host)
    C = np.zeros((4, NUM_GT), np.float64)
    for p in range(4):
        np.add.at(C[p], gt_idx, coef[:, p].astype(np.float64))

    in_maps = []
    for cidx in range(N_CORES):
        msel = slice(M_LOC * cidx, M_LOC * (cidx + 1))
        nsel = slice(NEG_LOC * cidx, NEG_LOC * (cidx + 1))
        coef_c = coef[msel]                                  # (32,4)
        w16 = np.zeros((16, 128), f32)
        for q in range(4):
            w16[4 * q:4 * q + 4, 32 * q:32 * q + 32] = coef_c.T
        zin = np.concatenate([proto16, w16.astype(bf16)], axis=1)
        small = np.zeros((128, 8), f32)
        small[:, 6] = 1.0
        small[:, 0] = np.concatenate([logit_pos[msel], logit_neg[nsel]])
        small[:, 1] = np.concatenate(
            [np.full(M_LOC, -1.0, f32), np.full(NEG_LOC, 1.0, f32)])
        # k-blocked loc packing: rows k*32 + j
        small[:, 2] = pred[msel].T.reshape(128)
        small[:, 3] = gt[msel].T.reshape(128)
        small[:, 4] = np.concatenate(
            [a_ch[msel], a_cw[msel], a_h[msel], a_w[msel]])
        small[:, 5] = np.concatenate(
            [a_h[msel], a_w[msel],
             np.full(M_LOC, LN10, f32), np.full(M_LOC, LN10, f32)])
        in_maps.append({
            "masks": np.ascontiguousarray(gt_masks[G_LOC * cidx:G_LOC * (cidx + 1)]),
            "zin": zin,
            "proto_cat": proto_cat,
            "small8": small,
        })
    return in_maps, C


def _combine(results, C):
    """results: list of per-core {'res': [128, NCOL]} dicts. float64 combine."""
    s_soft = 0.0
    s_cls = 0.0
    s_loc = 0.0
    s_dot = 0.0
    for cidx in range(N_CORES):
        rc = np.asarray(results[cidx]["res"], np.float64)
        s_soft += rc[:, 0:2].sum()
        s_cls += rc[:, 2].sum()
        s_loc += rc[:, 3].sum()
        for g in range(G_LOC):
            for p in range(4):
                s_dot += C[p, G_LOC * cidx + g] * rc[:, 4 + 4 * g + p].sum()
    total = s_cls + s_loc + (16.0 * s_soft - s_dot) / 262144.0 / float(M)
    return np.array(total, dtype=np.float32)


def kernel(**inputs):
    from concourse.bass_utils import run_bass_kernel_spmd
    nc = _get_nc()
    in_maps, C = _host_prep(inputs)
    out = run_bass_kernel_spmd(nc, in_maps, list(range(N_CORES)))
    return _combine(out.results, C)



# revision 12
# speedup vs baseline: 1.0489x; 1.0489x over previous
"""Trainium2 Bass kernel for nn_AllLoss_13400297964003.

Strategy (exact algebraic refactor of the reference loss):
  - The mask BCE term per anchor m is
        mean_{512x512}( softplus(up) - goal*up )
    with up = 4x nearest-upsample of z_m = coef_m . proto.  This equals
        ( 16*sum_ij softplus(z_m[ij]) - sum_ij z_m[ij]*G_m[ij] ) / 512^2
    where G_m = 4x4 block-sum pooling of gt_masks[gt_idx[m]].
  - The goal term collapses:  sum_m sum_ij z_m*G_m = sum_{p,g} C[p,g]*D[p,g]
    with C[p,g] = sum_{m: gt_idx[m]=g} coef[m,p]  (tiny, host-aggregated)
    and  D[p,g] = sum_ij proto[p,ij] * pool4x4(mask_g)[ij]  (device).
  - Sharding over 8 cores: core c gets anchors [32c,32c+32), gt masks
    [4c,4c+4), and 96 negative anchors.  Host combines scalars in float64.

v2 layout (vs the earlier SWDGE/f32r version):
  - masks ship as bf16 (exact for 0/1 data): 2.1MB/core instead of 4.2MB.
    All mask traffic rides the sync HWDGE queue as 8 half-mask DMAs
    (RTL descriptor gen, ~0.6us first-byte, FIFO back-to-back), so the
    stream starts immediately and pool matmuls pipeline per half-mask.
    Consts (zin / small4 / cat2) ride the scalar HWDGE queue.
  - row-pool via ONE shared [128,32] 0/1 weight: chunk c of a mask
    (raw rows 128c..128c+127 on partitions) matmuls into PSUM partitions
    32c..32c+31 -> R_g[128,512] holds all 128 pooled rows.  One LDWEIGHTS
    for all 16 pool matmuls, no accumulation chains, no on-device
    affine_select weight generation (weights come in the cat2 const DMA).
  - column-pool: DVE strided tensor_reduce [128,128,4] -> Pg bf16 (pool
    sums <=16 are exact in bf16); D partials via DVE multiply + segmented
    reduce against proto_cat bf16.
  - softplus is a single ACT pass (AF.Softplus) with accum_out, straight
    from PSUM, for both the mask term and the cls term -> exactly one
    activation table load, no Exp/Ln phase choreography.
  - loc smooth-L1 stays on device (DVE, f32); the encoded targets
    (including log10) are packed host-side so no Ln table is needed.
"""

import numpy as np

N_CORES = 8
M = 256
NUM_GT = 32
M_LOC = M // N_CORES          # 32 anchors per core
G_LOC = NUM_GT // N_CORES     # 4 gt masks per core
NEG_LOC = 3 * M // N_CORES    # 96 negative anchors per core
NCOL = 24                     # 0-3 softplus accums, 4 cls, 5 loc, 8..23 ddot

_CACHE = {}


def _build_nc():
    from contextlib import ExitStack
    import concourse.tile as tile
    from concourse import bacc, mybir
    from concourse.tile import add_dep_helper

    f32 = mybir.dt.float32
    bf16 = mybir.dt.bfloat16
    AF = mybir.ActivationFunctionType
    ALU = mybir.AluOpType
    AX = mybir.AxisListType

    nc = bacc.Bacc("TRN2", target_bir_lowering=False, debug=False)

    masks = nc.dram_tensor("masks", [G_LOC, 512, 512], bf16, kind="ExternalInput").ap()
    zin = nc.dram_tensor("zin", [16, 4224], bf16, kind="ExternalInput").ap()
    small4 = nc.dram_tensor("small4", [128, 4], f32, kind="ExternalInput").ap()
    cat2 = nc.dram_tensor("cat2", [128, 1024], bf16, kind="ExternalInput").ap()
    res = nc.dram_tensor("res", [128, NCOL], f32, kind="ExternalOutput").ap()

    with tile.TileContext(nc) as tc:
        with ExitStack() as ctx:
            constp = ctx.enter_context(tc.tile_pool(name="constp", bufs=1))
            maskp = ctx.enter_context(tc.tile_pool(name="maskp", bufs=4))
            zps = ctx.enter_context(tc.tile_pool(name="zps", bufs=3, space="PSUM"))
            rps = ctx.enter_context(tc.tile_pool(name="rps", bufs=2, space="PSUM"))
            workp = ctx.enter_context(tc.tile_pool(name="workp", bufs=3))
            outp = ctx.enter_context(tc.tile_pool(name="outp", bufs=1))

            # ---- const loads (scalar HWDGE queue) ----
            zin_t = constp.tile([16, 4224], bf16)
            nc.scalar.dma_start(zin_t[:], zin[:])
            proto16_t = zin_t[:, 0:4096]
            w16_t = zin_t[:, 4096:4224]
            small4_t = constp.tile([128, 4], f32)
            nc.scalar.dma_start(small4_t[:], small4[:])
            cat2_t = constp.tile([128, 1024], bf16)
            nc.scalar.dma_start(cat2_t[:], cat2[:])
            pc3 = cat2_t[:, 0:512].rearrange("p (a k) -> p a k", a=4)
            wpool_t = cat2_t[:, 512:544]          # [128, 32] 0/1 row-pool weight
            wpool3_t = cat2_t[:, 544:672]         # [128, 128] chunk-3 variant
            clsx_t = small4_t[:, 0:1]
            clssgn_t = small4_t[:, 1:2]
            locp_t = small4_t[:, 2:3]
            loct_t = small4_t[:, 3:4]

            PS = outp.tile([128, NCOL], f32)

            # ---- mask half DMAs (sync HWDGE, the dominant traffic) ----
            mts = []
            half_done = {}
            for g in range(G_LOC):
                t = maskp.tile([128, 4, 512], bf16, tag="mask")
                for h in (0, 1):
                    src = masks[g, 256 * h:256 * (h + 1), :].rearrange(
                        "(c p) J -> p c J", c=2)
                    half_done[(g, h)] = nc.sync.dma_start(t[:, 2 * h:2 * h + 2, :], src)
                mts.append(t)

            # ---- softplus = Ln(1 + Exp(x)), both from activation table 6
            # (natural_log_exp_and_others).  The auto table selector is
            # greedy (Exp would pick table 0, Ln table 5 -> thrash), so we
            # load table 6 explicitly once; the insert_act_table_loads pass
            # then sees every Exp/Ln already served and adds nothing. ----
            act_order = []
            tbl = nc.scalar.add_instruction(mybir.InstLoadActFuncSet(
                name=nc.get_next_instruction_name(), act_func_set_id=6))
            act_order.append(tbl)

            # cls: softplus(sign*logit), [128,1]
            cls_e = workp.tile([128, 1], f32, tag="cls_e")
            act_order.append(nc.scalar.activation(
                cls_e[:], clsx_t, AF.Exp, scale=clssgn_t))
            act_order.append(nc.scalar.activation(
                PS[:, 4:5], cls_e[:], AF.Ln, bias=1.0))

            # loc smooth-L1 head: d = pred - tgt (DVE), |d| (ACT, table 6)
            dve_order = []
            d = workp.tile([128, 1], f32, tag="sm1")
            dve_order.append(nc.vector.tensor_sub(d[:], locp_t, loct_t))
            a_t = workp.tile([128, 1], f32, tag="sm2")
            act_order.append(nc.scalar.activation(a_t[:], d[:], AF.Abs))

            # ---- z matmuls (bf16) -> Exp (PSUM->bf16) -> Ln(bias=1, accum) ----
            exm = [workp.tile([128, 1024], bf16, name=f"exm{i}",
                              tag=f"exm{i}") for i in range(2)]
            sp_scratch = [workp.tile([128, 1024], bf16, name=f"sps{i}",
                                     tag=f"sps{i}") for i in range(2)]
            zts = []
            z_mms = []
            pe_order = []
            for b in range(4):
                zt = zps.tile([128, 1024], f32, name=f"zt{b}", tag="z")
                zts.append(zt)
            for b in range(3):
                for half in range(2):
                    mm = nc.tensor.matmul(
                        zts[b][:, 512 * half:512 * (half + 1)], w16_t,
                        proto16_t[:, 1024 * b + 512 * half:1024 * b + 512 * (half + 1)],
                        start=True, stop=True)
                    z_mms.append(mm)
                    pe_order.append(mm)

            def softplus_block(b):
                act_order.append(nc.scalar.activation(
                    exm[b % 2][:], zts[b][:], AF.Exp))
                act_order.append(nc.scalar.activation(
                    sp_scratch[b % 2][:], exm[b % 2][:], AF.Ln, bias=1.0,
                    accum_out=PS[:, b:b + 1]))

            softplus_block(0)
            softplus_block(1)

            # ---- mask row-pool matmuls + per-mask DVE tail ----
            def pool_mask(g):
                R = rps.tile([128, 512], f32, tag="r")
                # matmul out base partition must be 0/32/64 (quadrant-3 HW
                # bug): chunks 0-2 use the shared [128,32] weight at their
                # partition offsets; chunk 3 uses a full-width weight at
                # base 0 with start=False (has_written bits fresh-write the
                # untouched partitions 96..127, written ones accumulate +0).
                for c in range(3):
                    mm = nc.tensor.matmul(
                        R[32 * c:32 * (c + 1), :], wpool_t,
                        mts[g][:, c, :],
                        start=True, stop=True)
                    pe_order.append(mm)
                mm = nc.tensor.matmul(
                    R[:], wpool3_t, mts[g][:, 3, :],
                    start=False, stop=True)
                pe_order.append(mm)
                r4 = R[:].rearrange("p (j four) -> p j four", four=4)
                Pg = workp.tile([128, 128], bf16, tag="Pg")
                with nc.allow_low_precision(
                        reason="pooled 0/1 mask sums <=16 are exact in bf16"):
                    dve_order.append(
                        nc.vector.tensor_reduce(Pg[:], r4, axis=AX.X, op=ALU.add))
                prod = workp.tile([128, 4, 128], bf16, tag="prod")
                pgb = Pg[:].unsqueeze(1).broadcast_to([128, 4, 128])
                dve_order.append(nc.vector.tensor_mul(prod[:], pgb, pc3))
                dve_order.append(
                    nc.vector.tensor_reduce(PS[:, 8 + 4 * g:12 + 4 * g], prod[:],
                                            axis=AX.X, op=ALU.add))

            pool_mask(0)
            # remaining z matmuls (tile reuse waits for softplus drains)
            for half in range(2):
                mm = nc.tensor.matmul(
                    zts[3][:, 512 * half:512 * (half + 1)], w16_t,
                    proto16_t[:, 3072 + 512 * half:3072 + 512 * (half + 1)],
                    start=True, stop=True)
                z_mms.append(mm)
                pe_order.append(mm)
            softplus_block(2)
            pool_mask(1)
            softplus_block(3)
            pool_mask(2)
            pool_mask(3)

            # ---- localization smooth-L1 tail (f32 DVE) ----
            mn = workp.tile([128, 1], f32, tag="sm3")
            dve_order.append(nc.vector.tensor_scalar(
                mn[:], a_t[:], 1.0, None, op0=ALU.min))
            amn = workp.tile([128, 1], f32, tag="sm4")
            dve_order.append(nc.vector.tensor_sub(amn[:], a_t[:], mn[:]))
            sq = workp.tile([128, 1], f32, tag="sm5")
            dve_order.append(nc.vector.tensor_mul(sq[:], mn[:], mn[:]))
            dve_order.append(nc.vector.scalar_tensor_tensor(
                PS[:, 5:6], sq[:], 0.5, amn[:], op0=ALU.mult, op1=ALU.add))

            # ---- ordering hints ----
            for a, b2 in zip(act_order, act_order[1:]):
                add_dep_helper(b2.ins, a.ins, sync=False, reason="act-order")
            for a, b2 in zip(pe_order, pe_order[1:]):
                add_dep_helper(b2.ins, a.ins, sync=False, reason="pe-order")
            for a, b2 in zip(dve_order, dve_order[1:]):
                add_dep_helper(b2.ins, a.ins, sync=False, reason="dve-order")

            # ---- write result ----
            nc.sync.dma_start(res[:], PS[:])

    nc.compile()
    return nc


def _get_nc():
    if "nc" not in _CACHE:
        _CACHE["nc"] = _build_nc()
    return _CACHE["nc"]


def _host_prep(inputs):
    """Pure index-driven gathers/packing. Returns per-core input maps plus
    the float64 C aggregation matrix used in the final scalar combine."""
    import ml_dtypes
    bf16 = ml_dtypes.bfloat16
    f32 = np.float32
    proto = np.asarray(inputs["proto_types"], f32)[0]        # (4,128,128)
    map_class = np.asarray(inputs["map_class"], f32)[0]      # (3,64,64)
    map_box = np.asarray(inputs["map_box"], f32)[0]          # (12,64,64)
    map_coef = np.asarray(inputs["map_coef"], f32)[0]        # (12,64,64)
    anchor_center = np.asarray(inputs["anchor_center"], f32)  # (2,64,64)
    anchor_box = np.asarray(inputs["anchor_box"], f32)       # (3,2)
    gt_boxes = np.asarray(inputs["gt_boxes"], f32)[0]        # (32,4)
    gt_masks = np.asarray(inputs["gt_masks"], f32)[0]        # (32,512,512)
    pos_idx = np.asarray(inputs["pos_idx"])
    gt_idx = np.asarray(inputs["gt_idx"])
    neg_idx = np.asarray(inputs["neg_idx"])

    r, hh, ww = pos_idx[:, 0], pos_idx[:, 1], pos_idx[:, 2]
    ch4 = r[:, None] * 4 + np.arange(4, dtype=r.dtype)[None, :]
    coef = map_coef[ch4, hh[:, None], ww[:, None]]           # (256,4)
    pred = map_box[ch4, hh[:, None], ww[:, None]]            # (256,4)
    logit_pos = map_class[r, hh, ww]                         # (256,)
    logit_neg = map_class[neg_idx[:, 0], neg_idx[:, 1], neg_idx[:, 2]]  # (768,)
    a_ch = anchor_center[0, hh, ww]
    a_cw = anchor_center[1, hh, ww]
    a_h = anchor_box[r, 0]
    a_w = anchor_box[r, 1]
    gt = gt_boxes[gt_idx]                                    # (256,4)
    # encoded loc targets (same f32 arithmetic as the reference)
    tgt = np.stack([(gt[:, 0] - a_ch) / a_h,
                    (gt[:, 1] - a_cw) / a_w,
                    np.log10(gt[:, 2] / a_h),
                    np.log10(gt[:, 3] / a_w)], axis=1).astype(f32)  # (256,4)

    # replicated tensors
    proto_flat = proto.reshape(4, 16384)
    proto16 = np.ascontiguousarray(
        proto_flat.reshape(4, 4, 4096).transpose(1, 0, 2).reshape(16, 4096)
    ).astype(bf16)
    proto_cat = np.ascontiguousarray(
        proto.transpose(1, 0, 2).reshape(128, 512)).astype(bf16)
    wpool = np.zeros((128, 32), f32)
    wpool[np.arange(128), np.arange(128) // 4] = 1.0
    wpool3 = np.zeros((128, 128), f32)
    wpool3[np.arange(128), 96 + np.arange(128) // 4] = 1.0
    cat2 = np.zeros((128, 1024), bf16)
    cat2[:, 0:512] = proto_cat
    cat2[:, 512:544] = wpool.astype(bf16)
    cat2[:, 544:672] = wpool3.astype(bf16)
    # C[p,g] aggregation (float64, host)
    C = np.zeros((4, NUM_GT), np.float64)
    for p in range(4):
        np.add.at(C[p], gt_idx, coef[:, p].astype(np.float64))

    masks_bf = gt_masks.astype(bf16)

    in_maps = []
    for cidx in range(N_CORES):
        msel = slice(M_LOC * cidx, M_LOC * (cidx + 1))
        nsel = slice(NEG_LOC * cidx, NEG_LOC * (cidx + 1))
        coef_c = coef[msel]                                  # (32,4)
        w16 = np.zeros((16, 128), f32)
        for q in range(4):
            w16[4 * q:4 * q + 4, 32 * q:32 * q + 32] = coef_c.T
        zin = np.concatenate([proto16, w16.astype(bf16)], axis=1)
        small = np.zeros((128, 4), f32)
        small[:, 0] = np.concatenate([logit_pos[msel], logit_neg[nsel]])
        small[:, 1] = np.concatenate(
            [np.full(M_LOC, -1.0, f32), np.full(NEG_LOC, 1.0, f32)])
        # k-blocked loc packing: rows k*32 + j
        small[:, 2] = pred[msel].T.reshape(128)
        small[:, 3] = tgt[msel].T.reshape(128)
        in_maps.append({
            "masks": np.ascontiguousarray(masks_bf[G_LOC * cidx:G_LOC * (cidx + 1)]),
            "zin": zin,
            "small4": small,
            "cat2": cat2,
        })
    return in_maps, C


def _combine(results, C):
    """results: list of per-core {'res': [128, NCOL]} dicts. float64 combine."""
    s_soft = 0.0
    s_cls = 0.0
    s_loc = 0.0
    s_dot = 0.0
    for cidx in range(N_CORES):
        rc = np.asarray(results[cidx]["res"], np.float64)
        s_soft += rc[:, 0:4].sum()
        s_cls += rc[:, 4].sum()
        s_loc += rc[:, 5].sum()
        for g in range(G_LOC):
            for p in range(4):
                s_dot += C[p, G_LOC * cidx + g] * rc[:, 8 + 4 * g + p].sum()
    total = s_cls + s_loc + (16.0 * s_soft - s_dot) / 262144.0 / float(M)
    return np.array(total, dtype=np.float32)


def kernel(**inputs):
    from concourse.bass_utils import run_bass_kernel_spmd
    nc = _get_nc()
    in_maps, C = _host_prep(inputs)
    out = run_bass_kernel_spmd(nc, in_maps, list(range(N_CORES)))
    return _combine(out.results, C)
